# revision 5
# baseline (speedup 1.0000x reference)
"""AdaptiveFusionDecoder Trainium2 kernel (8 NeuronCores, SPMD, no collectives).

Strategy:
  - All step-invariant matmuls hoisted out of the sequential GRU scan:
      X  = emb @ W_ihx.T          (per-token input projection, all steps)
      R3 = ret @ W_ihf.T,  D3 = (img-ret) @ W_ihf.T   (fusion-gate algebra:
          fused = ret + alpha*(img-ret)  =>  fused@W_ihf.T = R3 + alpha*D3)
      c_a = img@w_gi + ret@w_gr + gate_b  (constant part of the alpha logit)
  - Sequential part per step: gh = h @ W_hh.T (TensorE, bf16 weights
    stationary in SBUF), alpha matvec, and the GRU gate elementwise math.
  - Everything is kept "gate-major" (transposed: feature on partitions,
    batch on the free axis) so the recurrence needs no transposes at all.
  - Output projection logits = H_all @ out_W.T is vocab-sharded across the
    8 cores (4000 vocab columns each); the recurrence is replicated on all
    cores, so no inter-core communication is needed at all.
"""

import os

os.environ.setdefault("MYCRO_LOCAL_CACHE", "1")

import numpy as np
import ml_dtypes
from contextlib import ExitStack

import concourse.bass as bass
import concourse.bacc as bacc
import concourse.tile as tile
from concourse import mybir
from concourse.bass_utils import run_bass_kernel_spmd
from concourse.masks import make_identity

V, E, H = 32000, 512, 1024
B, S = 32, 64
SB = S * B            # 2048 rows in step-major order: j = s*B + b
NCORES = 8
VS = V // NCORES      # 4000 vocab columns per core
H3 = 3 * H            # 3072
KH = H // 128         # 8 contraction chunks over H
KE = E // 128         # 4 contraction chunks over E
MH3 = H3 // 128       # 24 output tiles over 3H
NB_E = 8              # vocab n-blocks in the logits phase
NE = VS // NB_E       # 500 columns per logits matmul
MT_E = SB // 128      # 16 m-tiles (of 4 steps x 32 batch) in logits phase

BF16 = mybir.dt.bfloat16
F32 = mybir.dt.float32
I32 = mybir.dt.int32
npbf16 = ml_dtypes.bfloat16
AF = mybir.ActivationFunctionType

_CACHE = {}
LAST_RESULT = None


def _cm(a):
    """[K, M] -> chunk-major [128, (K//128)*M]; slice [:, k*M+m0 : k*M+m1]
    is rows k*128..(k+1)*128 of `a`, cols m0:m1 (a TensorE lhsT tile)."""
    a = np.asarray(a)
    K, M = a.shape
    kc = K // 128
    assert kc * 128 == K
    return np.ascontiguousarray(
        a.reshape(kc, 128, M).transpose(1, 0, 2).reshape(128, kc * M)
    )


def _gm(v):
    """[K] -> gate-major [128, K//128]: out[p, c] = v[c*128+p]"""
    v = np.asarray(v)
    K = v.shape[0]
    return np.ascontiguousarray(v.reshape(K // 128, 128).T)


def _bc(ap_, pos, count):
    """Insert a stride-0 (broadcast) free dim at free-position `pos`."""
    l = [list(x) for x in ap_.ap]
    l.insert(pos + 1, [0, count])
    return bass.AP(tensor=ap_.tensor, offset=ap_.offset, ap=l)


def build():
    nc = bacc.Bacc()

    # ---- parameters (per-core) ----
    tok_idx_h = nc.declare_dram_parameter("tok_idx", [128, SB // 128], I32, isOutput=False)
    tokemb_h = nc.declare_dram_parameter("tok_embed", [V, E], F32, isOutput=False)
    imgT_h = nc.declare_dram_parameter("imgT", [128, KE * B], F32, isOutput=False)
    retT_h = nc.declare_dram_parameter("retT", [128, KE * B], F32, isOutput=False)
    wihx_h = nc.declare_dram_parameter("w_ihxT", [128, KE * H3], BF16, isOutput=False)
    wihf_h = nc.declare_dram_parameter("w_ihfT", [128, KE * H3], BF16, isOutput=False)
    whh_h = nc.declare_dram_parameter("w_hhT", [128, KH * H3], BF16, isOutput=False)
    ihw_h = nc.declare_dram_parameter("init_hWT", [128, KE * H], BF16, isOutput=False)
    ihb_h = nc.declare_dram_parameter("init_hbT", [128, KH], F32, isOutput=False)
    wgh_h = nc.declare_dram_parameter("w_ghT", [128, KH], BF16, isOutput=False)
    wgi_h = nc.declare_dram_parameter("w_giT", [128, KE], BF16, isOutput=False)
    wgr_h = nc.declare_dram_parameter("w_grT", [128, KE], BF16, isOutput=False)
    gateb_h = nc.declare_dram_parameter("gate_b", [1, 1], F32, isOutput=False)
    bih_h = nc.declare_dram_parameter("b_ihT", [128, MH3], F32, isOutput=False)
    bhh_h = nc.declare_dram_parameter("b_hhT", [128, MH3], F32, isOutput=False)
    outw_h = nc.declare_dram_parameter("outWT", [128, KH * VS], BF16, isOutput=False)
    outb_h = nc.declare_dram_parameter("outb", [1, VS], F32, isOutput=False)
    out_h = nc.declare_dram_parameter("out", [SB, VS], F32, isOutput=True)

    # internal DRAM scratch for XR^T, step-major: [64][128][24*32] bf16
    xrt_d = nc.dram_tensor("xrt", [S, 128, MH3 * B], BF16)

    with tile.TileContext(nc) as tc, ExitStack() as ctx:
        singles = ctx.enter_context(tc.tile_pool(name="singles", bufs=1))

        # ---- persistent SBUF tensors ----
        whh_sb = singles.tile([128, KH * H3], BF16)
        nc.sync.dma_start(out=whh_sb[:, :], in_=whh_h[:, :])
        h_all = singles.tile([128, KH, S + 1, B], BF16)   # h_t^T, slots 0..64
        embT = singles.tile([128, KE, SB], BF16)          # emb^T (gathered, transposed)
        D3T = singles.tile([128, MH3, B], BF16)
        R3b = singles.tile([128, MH3, B], F32)            # R3^T + b_ih (+ b_hh on r,z)
        c_aT = singles.tile([1, B], F32)
        outb_bc = singles.tile([128, VS], F32)

        imgT_sb = singles.tile([128, KE, B], F32)
        retT_sb = singles.tile([128, KE, B], F32)
        imgT_bf = singles.tile([128, KE, B], BF16)
        retT_bf = singles.tile([128, KE, B], BF16)
        dT_bf = singles.tile([128, KE, B], BF16)
        dT_sb = singles.tile([128, KE, B], F32)
        ident = singles.tile([128, 128], F32)
        tok_idx_sb = singles.tile([128, SB // 128], I32)
        ihb_sb = singles.tile([128, KH], F32)
        bih_sb = singles.tile([128, MH3], F32)
        bhh_sb = singles.tile([128, MH3], F32)
        wgh_sb = singles.tile([128, KH], BF16)
        wgi_sb = singles.tile([128, KE], BF16)
        wgr_sb = singles.tile([128, KE], BF16)
        gateb_sb = singles.tile([1, 1], F32)
        ones_bf = singles.tile([1, 128], BF16)

        nc.sync.dma_start(out=imgT_sb[:, :, :], in_=imgT_h[:, :].rearrange("p (c b) -> p c b", c=KE))
        nc.sync.dma_start(out=retT_sb[:, :, :], in_=retT_h[:, :].rearrange("p (c b) -> p c b", c=KE))
        nc.sync.dma_start(out=tok_idx_sb[:, :], in_=tok_idx_h[:, :])
        nc.sync.dma_start(out=ihb_sb[:, :], in_=ihb_h[:, :])
        nc.sync.dma_start(out=bih_sb[:, :], in_=bih_h[:, :])
        nc.sync.dma_start(out=bhh_sb[:, :], in_=bhh_h[:, :])
        nc.sync.dma_start(out=wgh_sb[:, :], in_=wgh_h[:, :])
        nc.sync.dma_start(out=wgi_sb[:, :], in_=wgi_h[:, :])
        nc.sync.dma_start(out=wgr_sb[:, :], in_=wgr_h[:, :])
        nc.sync.dma_start(out=gateb_sb[:, :], in_=gateb_h[:, :])
        # broadcast out_b to all partitions
        ob = outb_h[:, :]
        nc.sync.dma_start(
            out=outb_bc[:, :],
            in_=bass.AP(tensor=ob.tensor, offset=ob.offset, ap=[[0, 128], list(ob.ap[1])]),
        )
        make_identity(nc, ident[:, :])
        nc.vector.memset(ones_bf[:, :], 1.0)
        nc.vector.tensor_copy(imgT_bf[:, :, :], imgT_sb[:, :, :])
        nc.vector.tensor_copy(retT_bf[:, :, :], retT_sb[:, :, :])
        nc.vector.tensor_sub(dT_sb[:, :, :], imgT_sb[:, :, :], retT_sb[:, :, :])
        nc.vector.tensor_copy(dT_bf[:, :, :], dT_sb[:, :, :])

        # h_cur ping-pong pool (f32 hidden state for the gate math)
        hc = ctx.enter_context(tc.tile_pool(name="hc", bufs=2))

        # ======== Phase A: embedding gather + transpose ========
        with ExitStack() as actx:
            gat = actx.enter_context(tc.tile_pool(name="gat", bufs=3))
            psA = actx.enter_context(tc.tile_pool(name="psA", bufs=4, space="PSUM"))
            for g in range(SB // 128):
                emb_g = gat.tile([128, E], F32)
                nc.gpsimd.indirect_dma_start(
                    out=emb_g[:, :],
                    out_offset=None,
                    in_=tokemb_h[:, :],
                    in_offset=bass.IndirectOffsetOnAxis(ap=tok_idx_sb[:, g : g + 1], axis=0),
                )
                for c in range(KE):
                    pt = psA.tile([128, 128], F32)
                    nc.tensor.transpose(out=pt[:, :], in_=emb_g[:, c * 128 : (c + 1) * 128], identity=ident[:, :])
                    nc.scalar.activation(out=embT[:, c, g * 128 : (g + 1) * 128], in_=pt[:, :], func=AF.Copy)

        # ======== Phase C: h0, c_a, R3/D3 ========
        with ExitStack() as cctx:
            wihf_sb = cctx.enter_context(tc.tile_pool(name="wihf", bufs=1)).tile([128, KE * H3], BF16)
            nc.sync.dma_start(out=wihf_sb[:, :], in_=wihf_h[:, :])
            ihw_sb = cctx.enter_context(tc.tile_pool(name="ihw", bufs=1)).tile([128, KE * H], BF16)
            nc.sync.dma_start(out=ihw_sb[:, :], in_=ihw_h[:, :])
            psC1 = cctx.enter_context(tc.tile_pool(name="psC1", bufs=1, space="PSUM"))
            psC2 = cctx.enter_context(tc.tile_pool(name="psC2", bufs=1, space="PSUM"))
            psC3 = cctx.enter_context(tc.tile_pool(name="psC3", bufs=4, space="PSUM"))

            # h0 = tanh(init_h_W @ img^T + b)
            ph0 = psC1.tile([128, KH, B], F32)
            for m in range(KH):
                for k in range(KE):
                    nc.tensor.matmul(
                        out=ph0[:, m, :],
                        lhsT=ihw_sb[:, k * H + m * 128 : k * H + (m + 1) * 128],
                        rhs=imgT_bf[:, k, :],
                        start=(k == 0),
                        stop=(k == KE - 1),
                    )
            h_cur0 = hc.tile([128, KH, B], F32, tag="hcur")
            for m in range(KH):
                nc.scalar.activation(out=h_cur0[:, m, :], in_=ph0[:, m, :], func=AF.Tanh, bias=ihb_sb[:, m : m + 1])
            nc.scalar.activation(out=h_all[:, :, 0, :], in_=h_cur0[:, :, :], func=AF.Copy)

            # c_a = img @ w_gi + ret @ w_gr + gate_b
            pca = psC2.tile([1, B], F32)
            for k in range(KE):
                nc.tensor.matmul(out=pca[:, :], lhsT=wgi_sb[:, k : k + 1], rhs=imgT_bf[:, k, :], start=(k == 0), stop=False)
            for k in range(KE):
                nc.tensor.matmul(out=pca[:, :], lhsT=wgr_sb[:, k : k + 1], rhs=retT_bf[:, k, :], start=False, stop=(k == KE - 1))
            gb = gateb_sb[:, :]
            nc.vector.tensor_add(c_aT[:, :], pca[:, :], bass.AP(tensor=gb.tensor, offset=gb.offset, ap=[list(gb.ap[0]), [0, B]]))

            # R3^T and D3^T
            for m in range(MH3):
                pr = psC3.tile([128, B], F32, tag="psc3")
                for k in range(KE):
                    nc.tensor.matmul(
                        out=pr[:, :],
                        lhsT=wihf_sb[:, k * H3 + m * 128 : k * H3 + (m + 1) * 128],
                        rhs=retT_bf[:, k, :],
                        start=(k == 0),
                        stop=(k == KE - 1),
                    )
                nc.scalar.activation(out=R3b[:, m, :], in_=pr[:, :], func=AF.Copy)
                pd = psC3.tile([128, B], F32, tag="psc3")
                for k in range(KE):
                    nc.tensor.matmul(
                        out=pd[:, :],
                        lhsT=wihf_sb[:, k * H3 + m * 128 : k * H3 + (m + 1) * 128],
                        rhs=dT_bf[:, k, :],
                        start=(k == 0),
                        stop=(k == KE - 1),
                    )
                nc.scalar.activation(out=D3T[:, m, :], in_=pd[:, :], func=AF.Copy)
            # fold biases into R3b:  + b_ih everywhere, + b_hh on the r,z chunks
            nc.vector.tensor_add(R3b[:, :, :], R3b[:, :, :], _bc(bih_sb[:, :], 1, B))
            nc.vector.tensor_add(R3b[:, 0:16, :], R3b[:, 0:16, :], _bc(bhh_sb[:, 0:16], 1, B))

        # ======== Phase B: X precompute -> xrt_d (DRAM) ========
        with ExitStack() as bctx:
            wihx_sb = bctx.enter_context(tc.tile_pool(name="wihx", bufs=1)).tile([128, KE * H3], BF16)
            nc.sync.dma_start(out=wihx_sb[:, :], in_=wihx_h[:, :])
            stg_pool = bctx.enter_context(tc.tile_pool(name="stg", bufs=1))
            psB = bctx.enter_context(tc.tile_pool(name="psB", bufs=4, space="PSUM"))
            SBLK = 16  # steps per n-block
            for nb in range(S // SBLK):
                stg = stg_pool.tile([128, MH3, SBLK, B], BF16, tag="stg")
                for m in range(MH3):
                    px = psB.tile([128, SBLK * B], F32, tag="psb")
                    for k in range(KE):
                        nc.tensor.matmul(
                            out=px[:, :],
                            lhsT=wihx_sb[:, k * H3 + m * 128 : k * H3 + (m + 1) * 128],
                            rhs=embT[:, k, nb * SBLK * B : (nb + 1) * SBLK * B],
                            start=(k == 0),
                            stop=(k == KE - 1),
                        )
                    r3m = R3b[:, m, :]
                    nc.vector.tensor_add(
                        stg[:, m, :, :],
                        px[:, :].rearrange("p (s b) -> p s b", s=SBLK),
                        _bc(r3m, 0, SBLK),
                    )
                for s in range(SBLK):
                    nc.sync.dma_start(
                        out=xrt_d[nb * SBLK + s].rearrange("p (c b) -> p c b", c=MH3),
                        in_=stg[:, :, s, :],
                    )

        # ======== Phase D: recurrence ========
        xrt_pool = ctx.enter_context(tc.tile_pool(name="xrt", bufs=3))
        gp = ctx.enter_context(tc.tile_pool(name="gp", bufs=2))
        psD_gh = ctx.enter_context(tc.tile_pool(name="psDgh", bufs=2, space="PSUM"))
        psD_a = ctx.enter_context(tc.tile_pool(name="psDa", bufs=1, space="PSUM"))
        psD_ab = ctx.enter_context(tc.tile_pool(name="psDab", bufs=1, space="PSUM"))
        # logits-phase pools opened now so Tile may overlap E into D's gaps
        rhsE = ctx.enter_context(tc.tile_pool(name="rhsE", bufs=16))
        psE = ctx.enter_context(tc.tile_pool(name="psE", bufs=2, space="PSUM"))
        stE = ctx.enter_context(tc.tile_pool(name="stE", bufs=3))

        h_cur = h_cur0
        for t in range(S):
            xrt_t = xrt_pool.tile([128, MH3, B], BF16, tag="xrt")
            nc.sync.dma_start(out=xrt_t[:, :, :], in_=xrt_d[t].rearrange("p (c b) -> p c b", c=MH3))

            # alpha_t = sigmoid(h_t . w_gh + c_a)
            pa = psD_a.tile([1, B], F32, tag="pa")
            for k in range(KH):
                nc.tensor.matmul(out=pa[:, :], lhsT=wgh_sb[:, k : k + 1], rhs=h_all[:, k, t, :], start=(k == 0), stop=(k == KH - 1))
            s_a = gp.tile([1, B], F32, tag="sa")
            nc.vector.tensor_add(s_a[:, :], pa[:, :], c_aT[:, :])
            al = gp.tile([1, B], BF16, tag="al")
            nc.scalar.activation(out=al[:, :], in_=s_a[:, :], func=AF.Sigmoid)
            pab = psD_ab.tile([128, B], F32, tag="pab")
            nc.tensor.matmul(out=pab[:, :], lhsT=ones_bf[:, :], rhs=al[:, :], start=True, stop=True)
            al_bc = gp.tile([128, B], BF16, tag="albc")
            nc.scalar.activation(out=al_bc[:, :], in_=pab[:, :], func=AF.Copy)

            # gh^T = W_hh @ h_t^T  (+ accumulate over KH chunks)
            pgh = psD_gh.tile([128, MH3, B], F32, tag="pgh")
            for m in range(MH3):
                for k in range(KH):
                    nc.tensor.matmul(
                        out=pgh[:, m, :],
                        lhsT=whh_sb[:, k * H3 + m * 128 : k * H3 + (m + 1) * 128],
                        rhs=h_all[:, k, t, :],
                        start=(k == 0),
                        stop=(k == KH - 1),
                    )

            # gi = XR_t + alpha*D3
            u = gp.tile([128, MH3, B], BF16, tag="u")
            nc.vector.tensor_mul(u[:, :, :], D3T[:, :, :], _bc(al_bc[:, :], 0, MH3))
            gi = gp.tile([128, MH3, B], BF16, tag="gi")
            nc.vector.tensor_add(gi[:, :, :], u[:, :, :], xrt_t[:, :, :])

            # r,z = sigmoid(gi_rz + gh_rz)
            rz_s = gp.tile([128, 16, B], F32, tag="rzs")
            nc.vector.tensor_add(rz_s[:, :, :], pgh[:, 0:16, :], gi[:, 0:16, :])
            rz = gp.tile([128, 16, B], F32, tag="rz")
            nc.scalar.activation(out=rz[:, :, :], in_=rz_s[:, :, :], func=AF.Sigmoid)

            # n = tanh(gi_n + r*(gh_n + b_hh_n))
            hn = gp.tile([128, KH, B], F32, tag="hn")
            nc.vector.tensor_add(hn[:, :, :], pgh[:, 16:24, :], _bc(bhh_sb[:, 16:24], 1, B))
            t1 = gp.tile([128, KH, B], F32, tag="t1")
            nc.vector.tensor_mul(t1[:, :, :], rz[:, 0:8, :], hn[:, :, :])
            t2 = gp.tile([128, KH, B], F32, tag="t2")
            nc.vector.tensor_add(t2[:, :, :], t1[:, :, :], gi[:, 16:24, :])
            nn = gp.tile([128, KH, B], F32, tag="nn")
            nc.scalar.activation(out=nn[:, :, :], in_=t2[:, :, :], func=AF.Tanh)

            # h_new = n + z*(h - n)
            hmn = gp.tile([128, KH, B], F32, tag="hmn")
            nc.vector.tensor_sub(hmn[:, :, :], h_cur[:, :, :], nn[:, :, :])
            t3 = gp.tile([128, KH, B], F32, tag="t3")
            nc.vector.tensor_mul(t3[:, :, :], rz[:, 8:16, :], hmn[:, :, :])
            h_new = hc.tile([128, KH, B], F32, tag="hcur")
            nc.vector.tensor_add(h_new[:, :, :], nn[:, :, :], t3[:, :, :])
            nc.scalar.activation(out=h_all[:, :, t + 1, :], in_=h_new[:, :, :], func=AF.Copy)
            h_cur = h_new

        # ======== Phase E: logits ========
        for nb in range(NB_E):
            rhs_k = []
            for k in range(KH):
                rk = rhsE.tile([128, NE], BF16, tag="rhse")
                nc.sync.dma_start(out=rk[:, :], in_=outw_h[:, k * VS + nb * NE : k * VS + (nb + 1) * NE])
                rhs_k.append(rk)
            for m in range(MT_E):
                pe = psE.tile([128, NE], F32, tag="pse")
                for k in range(KH):
                    nc.tensor.matmul(
                        out=pe[:, :],
                        lhsT=h_all[:, k, 1 + m * 4 : 1 + (m + 1) * 4, :],
                        rhs=rhs_k[k][:, :],
                        start=(k == 0),
                        stop=(k == KH - 1),
                    )
                st = stE.tile([128, NE], F32, tag="ste")
                nc.vector.tensor_add(st[:, :], pe[:, :], outb_bc[:, nb * NE : (nb + 1) * NE])
                nc.sync.dma_start(out=out_h[m * 128 : (m + 1) * 128, nb * NE : (nb + 1) * NE], in_=st[:, :])

    nc.finalize()
    return nc


def _prep_inputs(inputs):
    inp = {k: np.asarray(v) for k, v in inputs.items()}
    tokens = inp["tokens_in"].astype(np.int32)                  # [B, S]
    tok_sm = np.ascontiguousarray(tokens.T).reshape(SB)         # j = s*B + b
    tok_idx = np.ascontiguousarray(tok_sm.reshape(SB // 128, 128).T)

    W_ih = inp["gru_W_ih"].astype(np.float32)                   # [3H, 2E]
    gw = inp["gate_W"].astype(np.float32)[0]                    # [H + 2E]

    common = {
        "tok_idx": tok_idx.astype(np.int32),
        "tok_embed": np.ascontiguousarray(inp["tok_embed"].astype(np.float32)),
        "imgT": _cm(inp["image_emb"].astype(np.float32).T),
        "retT": _cm(inp["retrieved_emb"].astype(np.float32).T),
        "w_ihxT": _cm(W_ih[:, :E].T).astype(npbf16),
        "w_ihfT": _cm(W_ih[:, E:].T).astype(npbf16),
        "w_hhT": _cm(inp["gru_W_hh"].astype(np.float32).T).astype(npbf16),
        "init_hWT": _cm(inp["init_h_W"].astype(np.float32).T).astype(npbf16),
        "init_hbT": _gm(inp["init_h_b"]).astype(np.float32),
        "w_ghT": _gm(gw[:H]).astype(npbf16),
        "w_giT": _gm(gw[H : H + E]).astype(npbf16),
        "w_grT": _gm(gw[H + E :]).astype(npbf16),
        "gate_b": inp["gate_b"].astype(np.float32).reshape(1, 1),
        "b_ihT": _gm(inp["gru_b_ih"]).astype(np.float32),
        "b_hhT": _gm(inp["gru_b_hh"]).astype(np.float32),
    }
    outW = inp["out_W"].astype(np.float32)
    outb = inp["out_b"].astype(np.float32)
    in_maps = []
    for c in range(NCORES):
        m = dict(common)
        m["outWT"] = _cm(np.ascontiguousarray(outW[c * VS : (c + 1) * VS].T)).astype(npbf16)
        m["outb"] = np.ascontiguousarray(outb[c * VS : (c + 1) * VS].reshape(1, VS))
        in_maps.append(m)
    return in_maps


def kernel(**inputs):
    global LAST_RESULT
    if "nc" not in _CACHE:
        _CACHE["nc"] = build()
    nc = _CACHE["nc"]
    in_maps = _prep_inputs(inputs)
    trace = bool(int(os.environ.get("KERNEL_TRACE", "0")))
    res = run_bass_kernel_spmd(nc, in_maps, core_ids=list(range(NCORES)), trace=trace)
    LAST_RESULT = res
    full = np.concatenate([np.asarray(res.results[c]["out"]) for c in range(NCORES)], axis=1)
    return np.ascontiguousarray(full.reshape(S, B, V).transpose(1, 0, 2)).astype(np.float32)


# revision 9
# speedup vs baseline: 1.4417x; 1.4417x over previous
"""AdaptiveFusionDecoder Trainium2 kernel (8 NeuronCores, SPMD, no collectives).

Strategy:
  - All step-invariant matmuls hoisted out of the sequential GRU scan:
      X  = emb @ W_ihx.T          (per-token input projection, all steps)
      R3 = ret @ W_ihf.T,  D3 = (img-ret) @ W_ihf.T   (fusion-gate algebra:
          fused = ret + alpha*(img-ret)  =>  fused@W_ihf.T = R3 + alpha*D3)
      c_a = img@w_gi + ret@w_gr + gate_b  (constant part of the alpha logit)
  - Sequential part per step: gh = h @ W_hh.T (TensorE, bf16 weights
    stationary in SBUF), alpha matvec, and the GRU gate elementwise math.
  - Everything is kept "gate-major" (transposed: feature on partitions,
    batch on the free axis) so the recurrence needs no transposes at all.
  - Output projection logits = H_all @ out_W.T is vocab-sharded across the
    8 cores (4000 vocab columns each); the recurrence is replicated on all
    cores, so no inter-core communication is needed at all.
  - The logits matmuls are interleaved into the recurrence (2 (m,nb) units
    per step once the needed h slots exist) so TensorE chews on them during
    the serial gate-math windows.
"""

import os

os.environ.setdefault("MYCRO_LOCAL_CACHE", "1")

import numpy as np
import ml_dtypes
from contextlib import ExitStack

import concourse.bass as bass
import concourse.bacc as bacc
import concourse.tile as tile
from concourse import mybir
from concourse.bass_utils import run_bass_kernel_spmd
from concourse.masks import make_identity

V, E, H = 32000, 512, 1024
B, S = 32, 64
SB = S * B            # 2048 rows in step-major order: j = s*B + b
NCORES = 8
VS = V // NCORES      # 4000 vocab columns per core
H3 = 3 * H            # 3072
KH = H // 128         # 8 contraction chunks over H
KE = E // 128         # 4 contraction chunks over E
MH3 = H3 // 128       # 24 output tiles over 3H
NB_E = 8              # vocab n-blocks in the logits phase
NE = VS // NB_E       # 500 columns per logits matmul
MT_E = SB // 128      # 16 m-tiles (of 4 steps x 32 batch) in logits phase

BF16 = mybir.dt.bfloat16
F32 = mybir.dt.float32
I32 = mybir.dt.int32
npbf16 = ml_dtypes.bfloat16
AF = mybir.ActivationFunctionType

_CACHE = {}
LAST_RESULT = None


def _cm(a):
    """[K, M] -> chunk-major [128, (K//128)*M]; slice [:, k*M+m0 : k*M+m1]
    is rows k*128..(k+1)*128 of `a`, cols m0:m1 (a TensorE lhsT tile)."""
    a = np.asarray(a)
    K, M = a.shape
    kc = K // 128
    assert kc * 128 == K
    return np.ascontiguousarray(
        a.reshape(kc, 128, M).transpose(1, 0, 2).reshape(128, kc * M)
    )


def _gm(v):
    """[K] -> gate-major [128, K//128]: out[p, c] = v[c*128+p]"""
    v = np.asarray(v)
    K = v.shape[0]
    return np.ascontiguousarray(v.reshape(K // 128, 128).T)


def _bc(ap_, pos, count):
    """Insert a stride-0 (broadcast) free dim at free-position `pos`."""
    l = [list(x) for x in ap_.ap]
    l.insert(pos + 1, [0, count])
    return bass.AP(tensor=ap_.tensor, offset=ap_.offset, ap=l)


def build():
    nc = bacc.Bacc()

    # ---- parameters (per-core) ----
    tok_idx_h = nc.declare_dram_parameter("tok_idx", [128, SB // 128], I32, isOutput=False)
    tokemb_h = nc.declare_dram_parameter("tok_embed", [V, E], F32, isOutput=False)
    imgT_h = nc.declare_dram_parameter("imgT", [128, KE * B], F32, isOutput=False)
    retT_h = nc.declare_dram_parameter("retT", [128, KE * B], F32, isOutput=False)
    wihx_h = nc.declare_dram_parameter("w_ihxT", [128, KE * H3], BF16, isOutput=False)
    wihf_h = nc.declare_dram_parameter("w_ihfT", [128, KE * H3], BF16, isOutput=False)
    whh_h = nc.declare_dram_parameter("w_hhT", [128, KH * H3], BF16, isOutput=False)
    ihw_h = nc.declare_dram_parameter("init_hWT", [128, KE * H], BF16, isOutput=False)
    ihb_h = nc.declare_dram_parameter("init_hbT", [128, KH], F32, isOutput=False)
    wgh_h = nc.declare_dram_parameter("w_ghT", [128, KH], BF16, isOutput=False)
    wgi_h = nc.declare_dram_parameter("w_giT", [128, KE], BF16, isOutput=False)
    wgr_h = nc.declare_dram_parameter("w_grT", [128, KE], BF16, isOutput=False)
    gateb_h = nc.declare_dram_parameter("gate_b", [1, 1], F32, isOutput=False)
    bih_h = nc.declare_dram_parameter("b_ihT", [128, MH3], F32, isOutput=False)
    bhh_h = nc.declare_dram_parameter("b_hhT", [128, MH3], F32, isOutput=False)
    outw_h = nc.declare_dram_parameter("outWT", [128, KH * VS], BF16, isOutput=False)
    outb_h = nc.declare_dram_parameter("outb", [1, VS], BF16, isOutput=False)
    out_h = nc.declare_dram_parameter("out", [SB, VS], F32, isOutput=True)

    # internal DRAM scratch for XR^T in 4-step blocks:
    # [S/4][128][(m:24)(s4:4)(b:32)] bf16, fully contiguous transfers
    xrt_d = nc.dram_tensor("xrt", [S // 4, 128, MH3 * 4 * B], BF16)

    with tile.TileContext(nc) as tc, ExitStack() as ctx:
        singles = ctx.enter_context(tc.tile_pool(name="singles", bufs=1))

        # ---- persistent SBUF tensors ----
        whh_sb = singles.tile([128, KH * H3], BF16)
        nc.sync.dma_start(out=whh_sb[:, :], in_=whh_h[:, :])
        h_all = singles.tile([128, KH, S + 1, B], BF16)   # h_t^T, slots 0..64
        D3T = singles.tile([128, MH3, B], BF16)
        R3b = singles.tile([128, MH3, B], F32)            # R3^T + b_ih (+ b_hh on r,z)
        c_aT = singles.tile([1, B], F32)

        imgT_sb = singles.tile([128, KE, B], F32)
        retT_sb = singles.tile([128, KE, B], F32)
        imgT_bf = singles.tile([128, KE, B], BF16)
        retT_bf = singles.tile([128, KE, B], BF16)
        dT_bf = singles.tile([128, KE, B], BF16)
        dT_sb = singles.tile([128, KE, B], F32)
        ident = singles.tile([128, 128], F32)
        tok_idx_sb = singles.tile([128, SB // 128], I32)
        ihb_sb = singles.tile([128, KH], F32)
        bih_sb = singles.tile([128, MH3], F32)
        bhh_sb = singles.tile([128, MH3], F32)
        bhhn_bf = singles.tile([128, KH], BF16)
        wgh_sb = singles.tile([128, KH], BF16)
        wgi_sb = singles.tile([128, KE], BF16)
        wgr_sb = singles.tile([128, KE], BF16)
        gateb_sb = singles.tile([1, 1], F32)
        ones_bf = singles.tile([1, 128], BF16)
        outbb = singles.tile([1, VS], BF16)

        nc.sync.dma_start(out=imgT_sb[:, :, :], in_=imgT_h[:, :].rearrange("p (c b) -> p c b", c=KE))
        nc.sync.dma_start(out=retT_sb[:, :, :], in_=retT_h[:, :].rearrange("p (c b) -> p c b", c=KE))
        nc.sync.dma_start(out=tok_idx_sb[:, :], in_=tok_idx_h[:, :])
        nc.sync.dma_start(out=ihb_sb[:, :], in_=ihb_h[:, :])
        nc.sync.dma_start(out=bih_sb[:, :], in_=bih_h[:, :])
        nc.sync.dma_start(out=bhh_sb[:, :], in_=bhh_h[:, :])
        nc.sync.dma_start(out=wgh_sb[:, :], in_=wgh_h[:, :])
        nc.sync.dma_start(out=wgi_sb[:, :], in_=wgi_h[:, :])
        nc.sync.dma_start(out=wgr_sb[:, :], in_=wgr_h[:, :])
        nc.sync.dma_start(out=gateb_sb[:, :], in_=gateb_h[:, :])
        nc.sync.dma_start(out=outbb[:, :], in_=outb_h[:, :])
        make_identity(nc, ident[:, :])
        nc.vector.memset(ones_bf[:, :], 1.0)
        nc.vector.tensor_copy(imgT_bf[:, :, :], imgT_sb[:, :, :])
        nc.vector.tensor_copy(retT_bf[:, :, :], retT_sb[:, :, :])
        nc.vector.tensor_sub(dT_sb[:, :, :], imgT_sb[:, :, :], retT_sb[:, :, :])
        nc.vector.tensor_copy(dT_bf[:, :, :], dT_sb[:, :, :])
        nc.vector.tensor_copy(bhhn_bf[:, :], bhh_sb[:, 16:24])

        # ======== Phases A+C+B (embT lives only here) ========
        abctx = ExitStack()
        embT = abctx.enter_context(tc.tile_pool(name="embp", bufs=1)).tile([128, KE, SB], BF16)

        # ======== Phase A: embedding gather + transpose ========
        with ExitStack() as actx:
            gat = actx.enter_context(tc.tile_pool(name="gat", bufs=3))
            psA = actx.enter_context(tc.tile_pool(name="psA", bufs=4, space="PSUM"))
            for g in range(SB // 128):
                emb_g = gat.tile([128, E], F32)
                nc.gpsimd.indirect_dma_start(
                    out=emb_g[:, :],
                    out_offset=None,
                    in_=tokemb_h[:, :],
                    in_offset=bass.IndirectOffsetOnAxis(ap=tok_idx_sb[:, g : g + 1], axis=0),
                )
                for c in range(KE):
                    pt = psA.tile([128, 128], F32)
                    nc.tensor.transpose(out=pt[:, :], in_=emb_g[:, c * 128 : (c + 1) * 128], identity=ident[:, :])
                    nc.scalar.activation(out=embT[:, c, g * 128 : (g + 1) * 128], in_=pt[:, :], func=AF.Copy)

        # ======== Phase C: h0, c_a, R3/D3 ========
        with ExitStack() as cctx:
            wihf_sb = cctx.enter_context(tc.tile_pool(name="wihf", bufs=1)).tile([128, KE * H3], BF16)
            nc.sync.dma_start(out=wihf_sb[:, :], in_=wihf_h[:, :])
            ihw_sb = cctx.enter_context(tc.tile_pool(name="ihw", bufs=1)).tile([128, KE * H], BF16)
            nc.sync.dma_start(out=ihw_sb[:, :], in_=ihw_h[:, :])
            psC1 = cctx.enter_context(tc.tile_pool(name="psC1", bufs=1, space="PSUM"))
            psC2 = cctx.enter_context(tc.tile_pool(name="psC2", bufs=1, space="PSUM"))
            psC3 = cctx.enter_context(tc.tile_pool(name="psC3", bufs=4, space="PSUM"))

            # h0 = tanh(init_h_W @ img^T + b)  -> h_all slot 0 (bf16)
            ph0 = psC1.tile([128, KH, B], F32)
            for m in range(KH):
                for k in range(KE):
                    nc.tensor.matmul(
                        out=ph0[:, m, :],
                        lhsT=ihw_sb[:, k * H + m * 128 : k * H + (m + 1) * 128],
                        rhs=imgT_bf[:, k, :],
                        start=(k == 0),
                        stop=(k == KE - 1),
                    )
            for m in range(KH):
                nc.scalar.activation(out=h_all[:, m, 0, :], in_=ph0[:, m, :], func=AF.Tanh, bias=ihb_sb[:, m : m + 1])

            # c_a = img @ w_gi + ret @ w_gr + gate_b
            pca = psC2.tile([1, B], F32)
            for k in range(KE):
                nc.tensor.matmul(out=pca[:, :], lhsT=wgi_sb[:, k : k + 1], rhs=imgT_bf[:, k, :], start=(k == 0), stop=False)
            for k in range(KE):
                nc.tensor.matmul(out=pca[:, :], lhsT=wgr_sb[:, k : k + 1], rhs=retT_bf[:, k, :], start=False, stop=(k == KE - 1))
            gb = gateb_sb[:, :]
            nc.vector.tensor_add(c_aT[:, :], pca[:, :], bass.AP(tensor=gb.tensor, offset=gb.offset, ap=[list(gb.ap[0]), [0, B]]))

            # R3^T and D3^T
            for m in range(MH3):
                pr = psC3.tile([128, B], F32, tag="psc3")
                for k in range(KE):
                    nc.tensor.matmul(
                        out=pr[:, :],
                        lhsT=wihf_sb[:, k * H3 + m * 128 : k * H3 + (m + 1) * 128],
                        rhs=retT_bf[:, k, :],
                        start=(k == 0),
                        stop=(k == KE - 1),
                    )
                nc.scalar.activation(out=R3b[:, m, :], in_=pr[:, :], func=AF.Copy)
                pd = psC3.tile([128, B], F32, tag="psc3")
                for k in range(KE):
                    nc.tensor.matmul(
                        out=pd[:, :],
                        lhsT=wihf_sb[:, k * H3 + m * 128 : k * H3 + (m + 1) * 128],
                        rhs=dT_bf[:, k, :],
                        start=(k == 0),
                        stop=(k == KE - 1),
                    )
                nc.scalar.activation(out=D3T[:, m, :], in_=pd[:, :], func=AF.Copy)
            # fold biases into R3b:  + b_ih everywhere, + b_hh on the r,z chunks
            nc.vector.tensor_add(R3b[:, :, :], R3b[:, :, :], _bc(bih_sb[:, :], 1, B))
            nc.vector.tensor_add(R3b[:, 0:16, :], R3b[:, 0:16, :], _bc(bhh_sb[:, 0:16], 1, B))

        # ======== Phase B: X precompute -> xrt_d (DRAM, 4-step blocks) ========
        with ExitStack() as bctx:
            wihx_sb = bctx.enter_context(tc.tile_pool(name="wihx", bufs=1)).tile([128, KE * H3], BF16)
            nc.sync.dma_start(out=wihx_sb[:, :], in_=wihx_h[:, :])
            stg_pool = bctx.enter_context(tc.tile_pool(name="stg", bufs=2))
            psB = bctx.enter_context(tc.tile_pool(name="psB", bufs=4, space="PSUM"))
            SBLK = 16  # steps per n-block
            for nb in range(S // SBLK):
                # staging layout [128][g:4][m:24][s4:4][b:32]
                stg = stg_pool.tile([128, 4, MH3, 4, B], BF16, tag="stg")
                for m in range(MH3):
                    px = psB.tile([128, SBLK * B], F32, tag="psb")
                    for k in range(KE):
                        nc.tensor.matmul(
                            out=px[:, :],
                            lhsT=wihx_sb[:, k * H3 + m * 128 : k * H3 + (m + 1) * 128],
                            rhs=embT[:, k, nb * SBLK * B : (nb + 1) * SBLK * B],
                            start=(k == 0),
                            stop=(k == KE - 1),
                        )
                    r3m = R3b[:, m, :]
                    nc.vector.tensor_add(
                        stg[:, :, m, :, :],
                        px[:, :].rearrange("p (g s b) -> p g s b", g=4, s=4),
                        _bc(_bc(r3m, 0, 4), 1, 4),
                    )
                for g in range(4):
                    nc.sync.dma_start(out=xrt_d[nb * 4 + g], in_=stg[:, g, :, :, :])
        abctx.close()

        # ======== outWT residency (after B's pools release) ========
        outw_sb = ctx.enter_context(tc.tile_pool(name="outw", bufs=1)).tile([128, KH * VS], BF16)
        nc.sync.dma_start(out=outw_sb[:, :], in_=outw_h[:, :])

        # ======== Phase D: recurrence with interleaved logits units ========
        xrt_pool = ctx.enter_context(tc.tile_pool(name="xrt", bufs=2))
        gp = ctx.enter_context(tc.tile_pool(name="gp", bufs=2))
        psD_gh = ctx.enter_context(tc.tile_pool(name="psDgh", bufs=2, space="PSUM"))
        psD_a = ctx.enter_context(tc.tile_pool(name="psDa", bufs=1, space="PSUM"))
        psD_ab = ctx.enter_context(tc.tile_pool(name="psDab", bufs=1, space="PSUM"))
        psE = ctx.enter_context(tc.tile_pool(name="psE", bufs=2, space="PSUM"))
        stE = ctx.enter_context(tc.tile_pool(name="stE", bufs=3))

        units = [(m, nb) for m in range(MT_E) for nb in range(NB_E)]
        ui = 0

        def emit_unit(m, nb):
            pe = psE.tile([128, NE], F32, tag="pse")
            for k in range(KH):
                nc.tensor.matmul(
                    out=pe[:, :],
                    lhsT=h_all[:, k, 1 + m * 4 : 1 + (m + 1) * 4, :],
                    rhs=outw_sb[:, k * VS + nb * NE : k * VS + (nb + 1) * NE],
                    start=(k == 0),
                    stop=False,
                )
            nc.tensor.matmul(
                out=pe[:, :], lhsT=ones_bf[:, :], rhs=outbb[:, nb * NE : (nb + 1) * NE],
                start=False, stop=True,
            )
            st = stE.tile([128, NE], F32, tag="ste")
            nc.scalar.activation(out=st[:, :], in_=pe[:, :], func=AF.Copy)
            nc.sync.dma_start(out=out_h[m * 128 : (m + 1) * 128, nb * NE : (nb + 1) * NE], in_=st[:, :])

        xrt4 = None
        for t in range(S):
            if t % 4 == 0:
                xrt4 = xrt_pool.tile([128, MH3, 4, B], BF16, tag="xrt")
                nc.sync.dma_start(
                    out=xrt4[:, :, :, :],
                    in_=xrt_d[t // 4].rearrange("p (m s b) -> p m s b", m=MH3, s=4),
                )

            # alpha_t = sigmoid(h_t . w_gh + c_a)
            pa = psD_a.tile([1, B], F32, tag="pa")
            for k in range(KH):
                nc.tensor.matmul(out=pa[:, :], lhsT=wgh_sb[:, k : k + 1], rhs=h_all[:, k, t, :], start=(k == 0), stop=(k == KH - 1))
            s_a = gp.tile([1, B], F32, tag="sa")
            nc.vector.tensor_add(s_a[:, :], pa[:, :], c_aT[:, :])
            al = gp.tile([1, B], BF16, tag="al")
            nc.scalar.activation(out=al[:, :], in_=s_a[:, :], func=AF.Sigmoid)
            pab = psD_ab.tile([128, B], F32, tag="pab")
            nc.tensor.matmul(out=pab[:, :], lhsT=ones_bf[:, :], rhs=al[:, :], start=True, stop=True)
            al_bc = gp.tile([128, B], BF16, tag="albc")
            nc.scalar.activation(out=al_bc[:, :], in_=pab[:, :], func=AF.Copy)

            # gh^T = W_hh @ h_t^T  (accumulate over KH chunks)
            pgh = psD_gh.tile([128, MH3, B], F32, tag="pgh")
            for m in range(MH3):
                for k in range(KH):
                    nc.tensor.matmul(
                        out=pgh[:, m, :],
                        lhsT=whh_sb[:, k * H3 + m * 128 : k * H3 + (m + 1) * 128],
                        rhs=h_all[:, k, t, :],
                        start=(k == 0),
                        stop=(k == KH - 1),
                    )

            # gi = XR_t + alpha*D3  (bf16)
            u = gp.tile([128, MH3, B], BF16, tag="u")
            nc.vector.tensor_mul(u[:, :, :], D3T[:, :, :], _bc(al_bc[:, :], 0, MH3))
            gi = gp.tile([128, MH3, B], BF16, tag="gi")
            nc.vector.tensor_add(gi[:, :, :], u[:, :, :], xrt4[:, :, t % 4, :])

            # r,z = sigmoid(gi_rz + gh_rz)
            rz_s = gp.tile([128, 16, B], BF16, tag="rzs")
            nc.vector.tensor_add(rz_s[:, :, :], pgh[:, 0:16, :], gi[:, 0:16, :])
            rz = gp.tile([128, 16, B], BF16, tag="rz")
            nc.scalar.activation(out=rz[:, :, :], in_=rz_s[:, :, :], func=AF.Sigmoid)

            # n = tanh(gi_n + r*(gh_n + b_hh_n))
            hn = gp.tile([128, KH, B], BF16, tag="hn")
            nc.vector.tensor_add(hn[:, :, :], pgh[:, 16:24, :], _bc(bhhn_bf[:, :], 1, B))
            t1 = gp.tile([128, KH, B], BF16, tag="t1")
            nc.vector.tensor_mul(t1[:, :, :], rz[:, 0:8, :], hn[:, :, :])
            t2 = gp.tile([128, KH, B], BF16, tag="t2")
            nc.vector.tensor_add(t2[:, :, :], t1[:, :, :], gi[:, 16:24, :])
            nn = gp.tile([128, KH, B], BF16, tag="nn")
            nc.scalar.activation(out=nn[:, :, :], in_=t2[:, :, :], func=AF.Tanh)

            # h_new = n + z*(h - n)  -> h_all slot t+1 (bf16)
            hmn = gp.tile([128, KH, B], BF16, tag="hmn")
            nc.vector.tensor_sub(hmn[:, :, :], h_all[:, :, t, :], nn[:, :, :])
            t3 = gp.tile([128, KH, B], BF16, tag="t3")
            nc.vector.tensor_mul(t3[:, :, :], rz[:, 8:16, :], hmn[:, :, :])
            nc.vector.tensor_add(h_all[:, :, t + 1, :], nn[:, :, :], t3[:, :, :])

            # interleave logits units whose h slots are final
            budget = 2 if t >= 5 else 0
            while budget > 0 and ui < len(units):
                m, nb = units[ui]
                if 4 * m + 4 > t:
                    break
                emit_unit(m, nb)
                ui += 1
                budget -= 1

        # tail: remaining logits units
        while ui < len(units):
            m, nb = units[ui]
            emit_unit(m, nb)
            ui += 1

    nc.finalize()
    return nc


def _prep_inputs(inputs):
    inp = {k: np.asarray(v) for k, v in inputs.items()}
    tokens = inp["tokens_in"].astype(np.int32)                  # [B, S]
    tok_sm = np.ascontiguousarray(tokens.T).reshape(SB)         # j = s*B + b
    tok_idx = np.ascontiguousarray(tok_sm.reshape(SB // 128, 128).T)

    W_ih = inp["gru_W_ih"].astype(np.float32)                   # [3H, 2E]
    gw = inp["gate_W"].astype(np.float32)[0]                    # [H + 2E]

    common = {
        "tok_idx": tok_idx.astype(np.int32),
        "tok_embed": np.ascontiguousarray(inp["tok_embed"].astype(np.float32)),
        "imgT": _cm(inp["image_emb"].astype(np.float32).T),
        "retT": _cm(inp["retrieved_emb"].astype(np.float32).T),
        "w_ihxT": _cm(W_ih[:, :E].T).astype(npbf16),
        "w_ihfT": _cm(W_ih[:, E:].T).astype(npbf16),
        "w_hhT": _cm(inp["gru_W_hh"].astype(np.float32).T).astype(npbf16),
        "init_hWT": _cm(inp["init_h_W"].astype(np.float32).T).astype(npbf16),
        "init_hbT": _gm(inp["init_h_b"]).astype(np.float32),
        "w_ghT": _gm(gw[:H]).astype(npbf16),
        "w_giT": _gm(gw[H : H + E]).astype(npbf16),
        "w_grT": _gm(gw[H + E :]).astype(npbf16),
        "gate_b": inp["gate_b"].astype(np.float32).reshape(1, 1),
        "b_ihT": _gm(inp["gru_b_ih"]).astype(np.float32),
        "b_hhT": _gm(inp["gru_b_hh"]).astype(np.float32),
    }
    outW = inp["out_W"].astype(np.float32)
    outb = inp["out_b"].astype(np.float32)
    in_maps = []
    for c in range(NCORES):
        m = dict(common)
        m["outWT"] = _cm(np.ascontiguousarray(outW[c * VS : (c + 1) * VS].T)).astype(npbf16)
        m["outb"] = np.ascontiguousarray(outb[c * VS : (c + 1) * VS].reshape(1, VS)).astype(npbf16)
        in_maps.append(m)
    return in_maps


def kernel(**inputs):
    global LAST_RESULT
    if "nc" not in _CACHE:
        _CACHE["nc"] = build()
    nc = _CACHE["nc"]
    in_maps = _prep_inputs(inputs)
    trace = bool(int(os.environ.get("KERNEL_TRACE", "0")))
    res = run_bass_kernel_spmd(nc, in_maps, core_ids=list(range(NCORES)), trace=trace)
    LAST_RESULT = res
    full = np.concatenate([np.asarray(res.results[c]["out"]) for c in range(NCORES)], axis=1)
    return np.ascontiguousarray(full.reshape(S, B, V).transpose(1, 0, 2)).astype(np.float32)
